# revision 22
# baseline (speedup 1.0000x reference)
"""Trainium2 Bass kernel for single-token MoE routing (nn_MixtureOfExperts_v2).

Problem:
    x [2304]; enc_top [256, 2304]; W_down [256, 64, 2304]; encoder_weights
    [256, 512, 64].
    codes = relu_offset(enc_top @ x)           (slope 0.0, offset 1/48)
    top4 values/indices of codes
    per selected expert i (gate v):
        s = W_down[i] @ x                      [64]
        c = relu_offset(E[i] @ s, slope 0.01)  [512]
        d = E[i]^T @ c                         [64]
        recon += W_down[i]^T @ d               [2304]
        recon += v * enc_top[i]
    output = recon                             [2304]

Distribution (8 cores, no collectives):
    Every core loads a replicated bf16 transposed copy of enc_top, computes
    all 256 codes on the PE, and runs top-4 on the vector engine
    (max_with_indices), so all cores agree on the routing.  Core c then
    processes selected slot (c % 4) alone: it gathers that expert's weights
    from a per-core table in HBM with one indirect DMA and runs the expert
    pipeline.  Cores c and c+4 process the same slot but emit complementary
    halves of the 2304-dim reconstruction (the per-core tables are built
    with the core's half of the input-dim chunks first, so the program is
    identical across cores - pure SPMD with per-core constants).  The host
    sums the 8 partial outputs (the cross-core reduction is a plain "+"
    done during unsharding).

Numerics: only the routing phase (codes -> top-4 indices) runs in bf16; the
top-4 gate values are recomputed in fp32 from gathered fp32 enc_top rows.
The entire expert pipeline is fp32 with fp32 PSUM accumulation.
"""

import os

import numpy as np
import ml_dtypes

import concourse.bacc as bacc
import concourse.bass as bass
import concourse.mybir as mybir
import concourse.tile as tile
from concourse.bass import IndirectOffsetOnAxis
from concourse.bass_utils import run_bass_kernel_spmd

# ---- problem constants (hardcoded per harness contract) ----
IN_DIM = 2304
SUB = 64
ATOMS = 512
NE = 256
K = 4
P = 128
NCHUNK = IN_DIM // P          # 18 chunks of 128 along input dim
HALF = NCHUNK // 2            # 9 chunks per core-half
ACHUNK = ATOMS // P           # 4 chunks of 128 along atoms
N_CORES = 8

W_COLS = NCHUNK * SUB         # 1152: W_down^T block (chunk-major, m innermost)
E_COLS = ACHUNK * SUB         # 256:  natural E block (atom-chunk-major)
R_COLS = NCHUNK               # 18:   enc_top row (chunk-major)
ET_COLS = ATOMS               # 512:  E^T block (rows 0..63 only, rest zero)
TABA_COLS = W_COLS + E_COLS + R_COLS + ET_COLS   # 1938
ET_OFF = W_COLS + E_COLS + R_COLS                # 1426
R_OFF = W_COLS + E_COLS                          # 1408

GRP = 6                       # enc_top chunks per DMA group
NGRP = NCHUNK // GRP
N_WARM = int(os.environ.get("KERNEL_WARM_MMS", "24"))

OFFSET = float(np.float32(1.0) / np.float32(48.0))  # 1/sqrt(2304), fp32

F32 = mybir.dt.float32
BF16 = mybir.dt.bfloat16
I32 = mybir.dt.int32
U32 = mybir.dt.uint32


def build_program():
    nc = bacc.Bacc("TRN2", target_bir_lowering=False, debug=False,
                   enable_partition_id=False)

    tabW = nc.dram_tensor("tabw", [NE * P, W_COLS], F32,
                          kind="ExternalInput")
    tabB = nc.dram_tensor("tabb", [NE * SUB, ATOMS], F32,
                          kind="ExternalInput")
    tabC = nc.dram_tensor("tabc", [NE * P, E_COLS + R_COLS], F32,
                          kind="ExternalInput")
    encbf = nc.dram_tensor("encbf", [P, NCHUNK, NE], BF16,
                           kind="ExternalInput")
    # fp32 consts: cols 0:18 x (partition-major chunks); col 18 row0-7: unused
    cf32 = nc.dram_tensor("cf32", [P, NCHUNK], F32, kind="ExternalInput")
    # bf16 consts: cols 0:18 x; cols 18:26 row 0: slot one-hot
    cbf16 = nc.dram_tensor("cbf16", [P, NCHUNK + 8], BF16,
                           kind="ExternalInput")
    out_d = nc.dram_tensor("out", [P, HALF], F32, kind="ExternalOutput")

    with tile.TileContext(nc) as tc:
        with (
            tc.tile_pool(name="sb", bufs=1) as sb,
            tc.tile_pool(name="enc", bufs=NGRP) as encp,
            tc.tile_pool(name="ps", bufs=1, space="PSUM") as ps,
        ):
            # ---- phase A: codes = enc_top @ x (bf16, PE) ----
            # enc group DMAs first on the sync (SP) queue; consts on scalar.
            enc_ts = []
            for g in range(NGRP):
                enc_t = encp.tile([P, GRP, NE], BF16, tag="enc")
                nc.sync.dma_start(enc_t[:], encbf[:, g * GRP:(g + 1) * GRP, :])
                enc_ts.append(enc_t)
            cb = sb.tile([P, NCHUNK + 8], BF16, tag="cbf")
            nc.scalar.dma_start(cb[:], cbf16[:])
            x_bf = cb[:, 0:NCHUNK]
            oh_bf = cb[0:1, NCHUNK:NCHUNK + 8]
            x_pm = sb.tile([P, NCHUNK], F32, tag="xpm")
            nc.scalar.dma_start(x_pm[:], cf32[:])

            # on-device constants
            ones_rbf = sb.tile([1, P], BF16, tag="onesrbf")
            nc.vector.memset(ones_rbf[:], 1.0)
            ones_c = sb.tile([P, 1], F32, tag="onesc")
            nc.vector.memset(ones_c[:], 1.0)

            codes_ps = ps.tile([1, NE], F32, tag="codes")
            for g in range(NGRP):
                for jo in range(GRP):
                    jj = g * GRP + jo
                    nc.tensor.matmul(
                        codes_ps[:],
                        lhsT=x_bf[:, jj:jj + 1],
                        rhs=enc_ts[g][:, jo, :],
                        start=(jj == 0),
                        stop=(jj == NCHUNK - 1),
                    )

            # ---- phase B: top-k (max8 on DVE, reading PSUM) + slot pick ----
            vals = sb.tile([1, 8], F32, tag="vals")
            idxs = sb.tile([1, 8], U32, tag="idxs")
            nc.vector.max_with_indices(vals[:], idxs[:], codes_ps[:])
            idxbf = sb.tile([1, 8], BF16, tag="idxbf")
            nc.vector.tensor_copy(idxbf[:], idxs[:])
            scr8 = sb.tile([1, 8], BF16, tag="scr8")
            nc.vector.tensor_tensor(
                out=scr8[:], in0=idxbf[:], in1=oh_bf,
                op=mybir.AluOpType.mult,
            )
            i_sel = sb.tile([1, 1], BF16, tag="isel")
            with nc.allow_low_precision(
                    reason="one-hot dot on small ints; exact in bf16"):
                nc.vector.tensor_reduce(
                    out=i_sel[:], in_=scr8[:], axis=mybir.AxisListType.X,
                    op=mybir.AluOpType.add,
                )
            # broadcast index to all partitions (bf16 single-pass matmul)
            ib_ps = ps.tile([P, 1], F32, tag="ib")
            nc.tensor.matmul(ib_ps[:], lhsT=ones_rbf[:], rhs=i_sel[:],
                             start=True, stop=True)
            iota_f = sb.tile([P, 1], F32, tag="iotaf")
            nc.gpsimd.iota(iota_f[:], pattern=[[0, 1]], base=0,
                           channel_multiplier=1,
                           allow_small_or_imprecise_dtypes=True)
            offa = sb.tile([P, 1], I32, tag="offa")
            nc.vector.scalar_tensor_tensor(
                out=offa[:], in0=ib_ps[:], scalar=float(P), in1=iota_f[:],
                op0=mybir.AluOpType.mult, op1=mybir.AluOpType.add,
            )
            offb = sb.tile([SUB, 1], I32, tag="offb")
            nc.vector.scalar_tensor_tensor(
                out=offb[:], in0=ib_ps[:SUB], scalar=float(SUB),
                in1=iota_f[:SUB],
                op0=mybir.AluOpType.mult, op1=mybir.AluOpType.add,
            )

            # ---- phase C: gather this slot's expert blocks ----
            # W first (the s-step long pole), then E^T (c-step), then
            # E-natural + enc_top row (dT / gate-value steps).
            gW = sb.tile([P, W_COLS], F32, tag="gw")
            nc.gpsimd.indirect_dma_start(
                out=gW[:], out_offset=None, in_=tabW[:],
                in_offset=IndirectOffsetOnAxis(ap=offa[:, :1], axis=0),
            )
            gB = sb.tile([SUB, ATOMS], F32, tag="gb")
            nc.gpsimd.indirect_dma_start(
                out=gB[:], out_offset=None, in_=tabB[:],
                in_offset=IndirectOffsetOnAxis(ap=offb[:, :1], axis=0),
            )
            gC = sb.tile([P, E_COLS + R_COLS], F32, tag="gc")
            nc.gpsimd.indirect_dma_start(
                out=gC[:], out_offset=None, in_=tabC[:],
                in_offset=IndirectOffsetOnAxis(ap=offa[:, :1], axis=0),
            )

            # ---- PE warm-keeper: junk matmuls spanning the topk+gather
            # gap so HAM doesn't re-throttle the PE before the expert
            # pipeline (results written to a scratch PSUM bank, unused) ----
            if N_WARM:
                junk_ps = ps.tile([1, NE], F32, tag="junk")
                for w in range(N_WARM):
                    nc.tensor.matmul(
                        junk_ps[:],
                        lhsT=x_bf[:, 0:1],
                        rhs=enc_ts[NGRP - 1][:, GRP - 1, :],
                        start=(w == 0),
                        stop=(w == N_WARM - 1),
                    )

            # ---- phase D: expert pipeline (fp32) ----
            # s = W @ x : accumulate over 18 chunks
            s_ps = ps.tile([SUB, 1], F32, tag="s")
            for jj in range(NCHUNK):
                nc.tensor.matmul(
                    s_ps[:],
                    lhsT=gW[:, jj * SUB:(jj + 1) * SUB],
                    rhs=x_pm[:, jj:jj + 1],
                    start=(jj == 0),
                    stop=(jj == NCHUNK - 1),
                )
            s_sb = sb.tile([SUB, 1], F32, tag="ssb")
            nc.vector.tensor_copy(s_sb[:], s_ps[:])

            # c = E @ s : 4 chunks of 128 atoms (lhsT = E^T slabs)
            c_ps = ps.tile([P, ACHUNK], F32, tag="c")
            for ck in range(ACHUNK):
                nc.tensor.matmul(
                    c_ps[:, ck:ck + 1],
                    lhsT=gB[:, ck * P:(ck + 1) * P],
                    rhs=s_sb[:],
                    start=True, stop=True,
                )
            # leaky relu with offset: c >= off ? c : 0.01*c
            cmask = sb.tile([P, ACHUNK], U32, tag="cmask")
            nc.vector.tensor_scalar(
                out=cmask[:], in0=c_ps[:], scalar1=OFFSET, scalar2=None,
                op0=mybir.AluOpType.is_ge,
            )
            cleak = sb.tile([P, ACHUNK], F32, tag="cleak")
            nc.vector.tensor_scalar(
                out=cleak[:], in0=c_ps[:], scalar1=0.01, scalar2=None,
                op0=mybir.AluOpType.mult,
            )
            c_relu = sb.tile([P, ACHUNK], F32, tag="crelu")
            nc.vector.select(c_relu[:], cmask[:], c_ps[:], cleak[:])

            # d^T = c^T @ E : accumulate 4 atom chunks -> [1, 64]
            dT_ps = ps.tile([1, SUB], F32, tag="dt")
            for ck in range(ACHUNK):
                nc.tensor.matmul(
                    dT_ps[:],
                    lhsT=c_relu[:, ck:ck + 1],
                    rhs=gC[:, ck * SUB:(ck + 1) * SUB],
                    start=(ck == 0),
                    stop=(ck == ACHUNK - 1),
                )

            # v = relu_offset(enc_top[i] . x) in fp32 (runs on DVE, parallel
            # with the PE chain above)
            vscr = sb.tile([P, NCHUNK], F32, tag="vscr")
            nc.vector.tensor_tensor(
                out=vscr[:], in0=gC[:, E_COLS:E_COLS + NCHUNK],
                in1=x_pm[:], op=mybir.AluOpType.mult,
            )
            vtmp = sb.tile([P, 1], F32, tag="vtmp")
            nc.vector.tensor_reduce(
                out=vtmp[:], in_=vscr[:], axis=mybir.AxisListType.X,
                op=mybir.AluOpType.add,
            )
            v_ps = ps.tile([1, 1], F32, tag="v")
            nc.tensor.matmul(v_ps[:], lhsT=vtmp[:], rhs=ones_c[:],
                             start=True, stop=True)
            vmask = sb.tile([1, 1], F32, tag="vmask")
            nc.vector.tensor_scalar(
                out=vmask[:], in0=v_ps[:], scalar1=OFFSET, scalar2=None,
                op0=mybir.AluOpType.is_ge,
            )

            # fused [d | v] broadcast to all partitions in one matmul pair
            dtv = sb.tile([1, SUB + 1], F32, tag="dtv")
            nc.vector.tensor_copy(dtv[:, 0:SUB], dT_ps[:])
            nc.vector.tensor_tensor(
                out=dtv[:, SUB:SUB + 1], in0=v_ps[:], in1=vmask[:],
                op=mybir.AluOpType.mult,
            )
            ones_r = sb.tile([1, P], F32, tag="onesr")
            nc.vector.memset(ones_r[:], 1.0)
            bb_ps = ps.tile([P, SUB + 1], F32, tag="bb")
            nc.tensor.matmul(bb_ps[:], lhsT=ones_r[:], rhs=dtv[:],
                             start=True, stop=True)

            # recon half: [128, 9] ; recon[p, jj] = sum_m W^T[p, jj, m]*d[m]
            prod = sb.tile([P, HALF, SUB], F32, tag="prod")
            gA_w3 = gW[:, 0:HALF * SUB].rearrange("p (j m) -> p j m", m=SUB)
            db_bc = bb_ps[:, None, 0:SUB].to_broadcast([P, HALF, SUB])
            nc.vector.tensor_tensor(
                out=prod[:], in0=gA_w3, in1=db_bc, op=mybir.AluOpType.mult,
            )
            recon = sb.tile([P, HALF], F32, tag="recon")
            nc.vector.tensor_reduce(
                out=recon[:], in_=prod[:], axis=mybir.AxisListType.X,
                op=mybir.AluOpType.add,
            )

            # final = recon + v * enc_row[:, :9]
            final = sb.tile([P, HALF], F32, tag="final")
            nc.vector.scalar_tensor_tensor(
                out=final[:],
                in0=gC[:, E_COLS:E_COLS + HALF],
                scalar=bb_ps[:, SUB:SUB + 1],
                in1=recon[:],
                op0=mybir.AluOpType.mult, op1=mybir.AluOpType.add,
            )
            nc.sync.dma_start(out_d[:], final[:])

    nc.compile()
    return nc


def _chunk_order(h):
    """Chunk visit order for core-half h: own half first."""
    own = list(range(h * HALF, (h + 1) * HALF))
    other = list(range((1 - h) * HALF, (2 - h) * HALF))
    return own + other


def _host_prep(x, enc_top, W_down, encoder_weights):
    """Build per-core-half input tables (pure layout transforms)."""
    x = np.asarray(x, np.float32)
    enc_top = np.asarray(enc_top, np.float32)
    W_down = np.asarray(W_down, np.float32)
    E = np.asarray(encoder_weights, np.float32)

    # natural-E block: rows g*128+p, cols ck*64+m = E[g, ck*128+p, m]
    encnat = np.ascontiguousarray(
        E.reshape(NE, ACHUNK, P, SUB).transpose(0, 2, 1, 3)
    ).reshape(NE * P, E_COLS)
    # E^T table: rows g*64+s, cols a = E[g, a, s]
    tabB = np.ascontiguousarray(E.transpose(0, 2, 1)).reshape(NE * SUB, ATOMS)

    Wr = W_down.reshape(NE, SUB, NCHUNK, P)          # [g, m, j, p]
    Er = enc_top.reshape(NE, NCHUNK, P)              # [g, j, p]

    per_half = {}
    for h in (0, 1):
        order = _chunk_order(h)
        tabW = np.ascontiguousarray(
            Wr[:, :, order, :].transpose(0, 3, 2, 1)  # [g, p, jj, m]
        ).reshape(NE * P, W_COLS)
        encrow = (
            Er[:, order, :].transpose(0, 2, 1)        # [g, p, jj]
        ).reshape(NE * P, R_COLS)
        tabC = np.concatenate([encnat, encrow], axis=1)

        x_pm = np.ascontiguousarray(
            x.reshape(NCHUNK, P)[order, :].T)          # [p, jj]
        encbf = np.ascontiguousarray(
            Er[:, order, :].transpose(2, 1, 0)         # [p, jj, g]
        ).astype(ml_dtypes.bfloat16)
        per_half[h] = dict(
            tabw=tabW,
            tabc=tabC,
            cf32=x_pm,
            xbf=x_pm.astype(ml_dtypes.bfloat16),
            encbf=encbf,
        )

    in_maps = []
    for c in range(N_CORES):
        h, slot = c // 4, c % 4
        ph = per_half[h]
        cbf = np.zeros((P, NCHUNK + 8), ml_dtypes.bfloat16)
        cbf[:, :NCHUNK] = ph["xbf"]
        cbf[0, NCHUNK + slot] = 1.0
        in_maps.append({
            "tabw": ph["tabw"],
            "tabb": tabB,
            "tabc": ph["tabc"],
            "encbf": ph["encbf"],
            "cf32": ph["cf32"],
            "cbf16": cbf,
        })
    return in_maps


def _assemble(results):
    out = np.zeros(IN_DIM, np.float32).reshape(NCHUNK, P)
    for c in range(N_CORES):
        h = c // 4
        own = _chunk_order(h)[:HALF]
        out[own, :] += results[c]["out"].T
    return out.reshape(IN_DIM)


_NC_CACHE = {}
LAST_RESULT = {}


def kernel(x, enc_top, W_down, encoder_weights):
    in_maps = _host_prep(x, enc_top, W_down, encoder_weights)
    if "nc" not in _NC_CACHE:
        _NC_CACHE["nc"] = build_program()
    nc = _NC_CACHE["nc"]

    if os.environ.get("BASS_SIM") == "1":
        from concourse.bass_interp import CoreSim
        sim_cores = os.environ.get("BASS_SIM_CORES")
        cores = (
            [int(t) for t in sim_cores.split(",")] if sim_cores
            else range(N_CORES)
        )
        results = [None] * N_CORES
        for c in cores:
            nc_c = build_program()
            sim = CoreSim(nc_c)
            for name, arr in in_maps[c].items():
                sim.tensor(name)[:] = arr
            sim.simulate()
            results[c] = {"out": np.array(sim.tensor("out"))}
        for c in range(N_CORES):
            if results[c] is None:
                results[c] = {"out": np.zeros((P, HALF), np.float32)}
        return _assemble(results)

    trace = os.environ.get("BASS_TRACE") == "1"
    if trace:
        _ensure_trace_hook()
    res = run_bass_kernel_spmd(
        nc, in_maps, core_ids=list(range(N_CORES)),
        trace=trace,
    )
    LAST_RESULT["res"] = res
    return _assemble(res.results)


def _ensure_trace_hook():
    """Install the axon NTFF profile hook if antenv.axon_hooks is absent."""
    try:
        from antenv.axon_hooks import get_axon_ntff_profile_hook  # noqa
        return
    except ImportError:
        pass
    import sys
    import types
    try:
        from trn_agent_boot.trn_boot import _ntff_profile_via_ctypes
    except ImportError:
        return
    hook = _ntff_profile_via_ctypes("/opt/axon/libaxon_pjrt.so")
    mod = types.ModuleType("antenv.axon_hooks")
    mod._hook = hook
    mod.get_axon_ntff_profile_hook = lambda: mod._hook
    mod.set_axon_ntff_profile_hook = lambda h: setattr(mod, "_hook", h)
    import antenv
    sys.modules["antenv.axon_hooks"] = mod
    antenv.axon_hooks = mod


if __name__ == "__main__":
    nc = build_program()
    print("program built ok")


# revision 27
# speedup vs baseline: 1.0278x; 1.0278x over previous
"""Trainium2 Bass kernel for single-token MoE routing (nn_MixtureOfExperts_v2).

Problem:
    x [2304]; enc_top [256, 2304]; W_down [256, 64, 2304]; encoder_weights
    [256, 512, 64].
    codes = relu_offset(enc_top @ x)           (slope 0.0, offset 1/48)
    top4 values/indices of codes
    per selected expert i (gate v):
        s = W_down[i] @ x                      [64]
        c = relu_offset(E[i] @ s, slope 0.01)  [512]
        d = E[i]^T @ c                         [64]
        recon += W_down[i]^T @ d               [2304]
        recon += v * enc_top[i]
    output = recon                             [2304]

Distribution (8 cores, no collectives):
    Every core loads a replicated bf16 transposed copy of enc_top, computes
    all 256 codes on the PE, and runs top-4 on the vector engine
    (max_with_indices), so all cores agree on the routing.  Core c then
    processes selected slot (c % 4) alone: it gathers that expert's weights
    from a per-core table in HBM with one indirect DMA and runs the expert
    pipeline.  Cores c and c+4 process the same slot but emit complementary
    halves of the 2304-dim reconstruction (the per-core tables are built
    with the core's half of the input-dim chunks first, so the program is
    identical across cores - pure SPMD with per-core constants).  The host
    sums the 8 partial outputs (the cross-core reduction is a plain "+"
    done during unsharding).

Numerics: only the routing phase (codes -> top-4 indices) runs in bf16; the
top-4 gate values are recomputed in fp32 from gathered fp32 enc_top rows.
The entire expert pipeline is fp32 with fp32 PSUM accumulation.
"""

import os

import numpy as np
import ml_dtypes

import concourse.bacc as bacc
import concourse.bass as bass
import concourse.mybir as mybir
import concourse.tile as tile
from concourse.bass import IndirectOffsetOnAxis
from concourse.bass_utils import run_bass_kernel_spmd

# ---- problem constants (hardcoded per harness contract) ----
IN_DIM = 2304
SUB = 64
ATOMS = 512
NE = 256
K = 4
P = 128
NCHUNK = IN_DIM // P          # 18 chunks of 128 along input dim
HALF = NCHUNK // 2            # 9 chunks per core-half
ACHUNK = ATOMS // P           # 4 chunks of 128 along atoms
N_CORES = 8

W_COLS = NCHUNK * SUB         # 1152: W_down^T block (chunk-major, m innermost)
E_COLS = ACHUNK * SUB         # 256:  natural E block (atom-chunk-major)
R_COLS = NCHUNK               # 18:   enc_top row (chunk-major)
ET_COLS = ATOMS               # 512:  E^T block (rows 0..63 only, rest zero)
TABA_COLS = W_COLS + E_COLS + R_COLS + ET_COLS   # 1938
ET_OFF = W_COLS + E_COLS + R_COLS                # 1426
R_OFF = W_COLS + E_COLS                          # 1408

ENC_GROUPS = [3, 6, 9]        # enc_top chunks per DMA group (first smallest
NGRP = len(ENC_GROUPS)        # so the PE can start earliest)
N_WARM = int(os.environ.get("KERNEL_WARM_MMS", "48"))

OFFSET = float(np.float32(1.0) / np.float32(48.0))  # 1/sqrt(2304), fp32

F32 = mybir.dt.float32
BF16 = mybir.dt.bfloat16
I32 = mybir.dt.int32
U32 = mybir.dt.uint32


def build_program():
    nc = bacc.Bacc("TRN2", target_bir_lowering=False, debug=False,
                   enable_partition_id=False)

    tabW = nc.dram_tensor("tabw", [NE * P, W_COLS], F32,
                          kind="ExternalInput")
    tabB = nc.dram_tensor("tabb", [NE * SUB, ATOMS], F32,
                          kind="ExternalInput")
    tabC = nc.dram_tensor("tabc", [NE * P, E_COLS + R_COLS], F32,
                          kind="ExternalInput")
    encbf = nc.dram_tensor("encbf", [P, NCHUNK, NE], BF16,
                           kind="ExternalInput")
    # fp32 consts: cols 0:18 x (partition-major chunks); col 18 row0-7: unused
    cf32 = nc.dram_tensor("cf32", [P, NCHUNK], F32, kind="ExternalInput")
    # bf16 consts: cols 0:18 x; cols 18:26 row 0: slot one-hot
    cbf16 = nc.dram_tensor("cbf16", [P, NCHUNK + 8], BF16,
                           kind="ExternalInput")
    out_d = nc.dram_tensor("out", [P, HALF], F32, kind="ExternalOutput")

    with tile.TileContext(nc) as tc:
        with (
            tc.tile_pool(name="sb", bufs=1) as sb,
            tc.tile_pool(name="enc", bufs=1) as encp,
            tc.tile_pool(name="ps", bufs=1, space="PSUM") as ps,
        ):
            # ---- phase A: codes = enc_top @ x (bf16, PE) ----
            # first (smallest) enc group + consts on the scalar (ACT) queue,
            # remaining groups on the sync (SP) queue - parallel issue.
            cb = sb.tile([P, NCHUNK + 8], BF16, tag="cbf")
            nc.scalar.dma_start(cb[:], cbf16[:])
            x_bf = cb[:, 0:NCHUNK]
            oh_bf = cb[0:1, NCHUNK:NCHUNK + 8]
            enc_ts = []
            g0 = 0
            for gi, gn in enumerate(ENC_GROUPS):
                enc_t = encp.tile([P, gn, NE], BF16, tag=f"enc{gi}")
                eng = nc.scalar if gi == 0 else nc.sync
                eng.dma_start(enc_t[:], encbf[:, g0:g0 + gn, :])
                enc_ts.append((enc_t, g0, gn))
                g0 += gn
            x_pm = sb.tile([P, NCHUNK], F32, tag="xpm")
            nc.scalar.dma_start(x_pm[:], cf32[:])

            # on-device constants
            ones_rbf = sb.tile([1, P], BF16, tag="onesrbf")
            nc.vector.memset(ones_rbf[:], 1.0)
            ones_c = sb.tile([P, 1], F32, tag="onesc")
            nc.vector.memset(ones_c[:], 1.0)

            codes_ps = ps.tile([1, NE], F32, tag="codes")
            for enc_t, g0, gn in enc_ts:
                for jo in range(gn):
                    jj = g0 + jo
                    nc.tensor.matmul(
                        codes_ps[:],
                        lhsT=x_bf[:, jj:jj + 1],
                        rhs=enc_t[:, jo, :],
                        start=(jj == 0),
                        stop=(jj == NCHUNK - 1),
                    )

            # ---- phase B: top-k (max8 on DVE, reading PSUM) + slot pick ----
            vals = sb.tile([1, 8], F32, tag="vals")
            idxs = sb.tile([1, 8], U32, tag="idxs")
            nc.vector.max_with_indices(vals[:], idxs[:], codes_ps[:])
            idxbf = sb.tile([1, 8], BF16, tag="idxbf")
            nc.vector.tensor_copy(idxbf[:], idxs[:])
            scr8 = sb.tile([1, 8], BF16, tag="scr8")
            nc.vector.tensor_tensor(
                out=scr8[:], in0=idxbf[:], in1=oh_bf,
                op=mybir.AluOpType.mult,
            )
            i_sel = sb.tile([1, 1], BF16, tag="isel")
            with nc.allow_low_precision(
                    reason="one-hot dot on small ints; exact in bf16"):
                nc.vector.tensor_reduce(
                    out=i_sel[:], in_=scr8[:], axis=mybir.AxisListType.X,
                    op=mybir.AluOpType.add,
                )
            # broadcast index to all partitions (bf16 single-pass matmul)
            ib_ps = ps.tile([P, 1], F32, tag="ib")
            nc.tensor.matmul(ib_ps[:], lhsT=ones_rbf[:], rhs=i_sel[:],
                             start=True, stop=True)
            iota_f = sb.tile([P, 1], F32, tag="iotaf")
            nc.gpsimd.iota(iota_f[:], pattern=[[0, 1]], base=0,
                           channel_multiplier=1,
                           allow_small_or_imprecise_dtypes=True)
            offa = sb.tile([P, 1], I32, tag="offa")
            nc.vector.scalar_tensor_tensor(
                out=offa[:], in0=ib_ps[:], scalar=float(P), in1=iota_f[:],
                op0=mybir.AluOpType.mult, op1=mybir.AluOpType.add,
            )
            offb = sb.tile([SUB, 1], I32, tag="offb")
            nc.vector.scalar_tensor_tensor(
                out=offb[:], in0=ib_ps[:SUB], scalar=float(SUB),
                in1=iota_f[:SUB],
                op0=mybir.AluOpType.mult, op1=mybir.AluOpType.add,
            )

            # ---- phase C: gather this slot's expert blocks ----
            # W first (the s-step long pole), then E^T (c-step), then
            # E-natural + enc_top row (dT / gate-value steps).
            gW = sb.tile([P, W_COLS], F32, tag="gw")
            nc.gpsimd.indirect_dma_start(
                out=gW[:], out_offset=None, in_=tabW[:],
                in_offset=IndirectOffsetOnAxis(ap=offa[:, :1], axis=0),
            )
            gB = sb.tile([SUB, ATOMS], F32, tag="gb")
            nc.gpsimd.indirect_dma_start(
                out=gB[:], out_offset=None, in_=tabB[:],
                in_offset=IndirectOffsetOnAxis(ap=offb[:, :1], axis=0),
            )
            gC = sb.tile([P, E_COLS + R_COLS], F32, tag="gc")
            nc.gpsimd.indirect_dma_start(
                out=gC[:], out_offset=None, in_=tabC[:],
                in_offset=IndirectOffsetOnAxis(ap=offa[:, :1], axis=0),
            )

            # ---- PE warm-keeper: junk matmuls spanning the topk+gather
            # gap so HAM doesn't re-throttle the PE before the expert
            # pipeline (results written to a scratch PSUM bank, unused) ----
            if N_WARM:
                junk_ps = ps.tile([1, NE], F32, tag="junk")
                last_t, _, last_n = enc_ts[-1]
                for w in range(N_WARM):
                    nc.tensor.matmul(
                        junk_ps[:],
                        lhsT=x_bf[:, 0:1],
                        rhs=last_t[:, last_n - 1, :],
                        start=(w == 0),
                        stop=(w == N_WARM - 1),
                    )

            # ---- phase D: expert pipeline (fp32) ----
            # s = W @ x : accumulate over 18 chunks
            s_ps = ps.tile([SUB, 1], F32, tag="s")
            for jj in range(NCHUNK):
                nc.tensor.matmul(
                    s_ps[:],
                    lhsT=gW[:, jj * SUB:(jj + 1) * SUB],
                    rhs=x_pm[:, jj:jj + 1],
                    start=(jj == 0),
                    stop=(jj == NCHUNK - 1),
                )
            s_sb = sb.tile([SUB, 1], F32, tag="ssb")
            nc.vector.tensor_copy(s_sb[:], s_ps[:])

            # c = E @ s : 4 chunks of 128 atoms (lhsT = E^T slabs)
            c_ps = ps.tile([P, ACHUNK], F32, tag="c")
            for ck in range(ACHUNK):
                nc.tensor.matmul(
                    c_ps[:, ck:ck + 1],
                    lhsT=gB[:, ck * P:(ck + 1) * P],
                    rhs=s_sb[:],
                    start=True, stop=True,
                )
            # leaky relu with offset: c >= off ? c : 0.01*c
            cmask = sb.tile([P, ACHUNK], U32, tag="cmask")
            nc.vector.tensor_scalar(
                out=cmask[:], in0=c_ps[:], scalar1=OFFSET, scalar2=None,
                op0=mybir.AluOpType.is_ge,
            )
            cleak = sb.tile([P, ACHUNK], F32, tag="cleak")
            nc.vector.tensor_scalar(
                out=cleak[:], in0=c_ps[:], scalar1=0.01, scalar2=None,
                op0=mybir.AluOpType.mult,
            )
            c_relu = sb.tile([P, ACHUNK], F32, tag="crelu")
            nc.vector.select(c_relu[:], cmask[:], c_ps[:], cleak[:])

            # d^T = c^T @ E : accumulate 4 atom chunks -> [1, 64]
            dT_ps = ps.tile([1, SUB], F32, tag="dt")
            for ck in range(ACHUNK):
                nc.tensor.matmul(
                    dT_ps[:],
                    lhsT=c_relu[:, ck:ck + 1],
                    rhs=gC[:, ck * SUB:(ck + 1) * SUB],
                    start=(ck == 0),
                    stop=(ck == ACHUNK - 1),
                )

            # v = relu_offset(enc_top[i] . x) in fp32 (runs on DVE, parallel
            # with the PE chain above)
            vscr = sb.tile([P, NCHUNK], F32, tag="vscr")
            nc.vector.tensor_tensor(
                out=vscr[:], in0=gC[:, E_COLS:E_COLS + NCHUNK],
                in1=x_pm[:], op=mybir.AluOpType.mult,
            )
            vtmp = sb.tile([P, 1], F32, tag="vtmp")
            nc.vector.tensor_reduce(
                out=vtmp[:], in_=vscr[:], axis=mybir.AxisListType.X,
                op=mybir.AluOpType.add,
            )
            v_ps = ps.tile([1, 1], F32, tag="v")
            nc.tensor.matmul(v_ps[:], lhsT=vtmp[:], rhs=ones_c[:],
                             start=True, stop=True)
            vmask = sb.tile([1, 1], F32, tag="vmask")
            nc.vector.tensor_scalar(
                out=vmask[:], in0=v_ps[:], scalar1=OFFSET, scalar2=None,
                op0=mybir.AluOpType.is_ge,
            )

            # fused [d | v] broadcast to all partitions in one matmul pair
            dtv = sb.tile([1, SUB + 1], F32, tag="dtv")
            nc.vector.tensor_copy(dtv[:, 0:SUB], dT_ps[:])
            nc.vector.tensor_tensor(
                out=dtv[:, SUB:SUB + 1], in0=v_ps[:], in1=vmask[:],
                op=mybir.AluOpType.mult,
            )
            ones_r = sb.tile([1, P], F32, tag="onesr")
            nc.vector.memset(ones_r[:], 1.0)
            bb_ps = ps.tile([P, SUB + 1], F32, tag="bb")
            nc.tensor.matmul(bb_ps[:], lhsT=ones_r[:], rhs=dtv[:],
                             start=True, stop=True)

            # recon half: [128, 9] ; recon[p, jj] = sum_m W^T[p, jj, m]*d[m]
            prod = sb.tile([P, HALF, SUB], F32, tag="prod")
            gA_w3 = gW[:, 0:HALF * SUB].rearrange("p (j m) -> p j m", m=SUB)
            db_bc = bb_ps[:, None, 0:SUB].to_broadcast([P, HALF, SUB])
            nc.vector.tensor_tensor(
                out=prod[:], in0=gA_w3, in1=db_bc, op=mybir.AluOpType.mult,
            )
            recon = sb.tile([P, HALF], F32, tag="recon")
            nc.vector.tensor_reduce(
                out=recon[:], in_=prod[:], axis=mybir.AxisListType.X,
                op=mybir.AluOpType.add,
            )

            # final = recon + v * enc_row[:, :9]
            final = sb.tile([P, HALF], F32, tag="final")
            nc.vector.scalar_tensor_tensor(
                out=final[:],
                in0=gC[:, E_COLS:E_COLS + HALF],
                scalar=bb_ps[:, SUB:SUB + 1],
                in1=recon[:],
                op0=mybir.AluOpType.mult, op1=mybir.AluOpType.add,
            )
            nc.sync.dma_start(out_d[:], final[:])

    nc.compile()
    return nc


def _chunk_order(h):
    """Chunk visit order for core-half h: own half first."""
    own = list(range(h * HALF, (h + 1) * HALF))
    other = list(range((1 - h) * HALF, (2 - h) * HALF))
    return own + other


def _host_prep(x, enc_top, W_down, encoder_weights):
    """Build per-core-half input tables (pure layout transforms)."""
    x = np.asarray(x, np.float32)
    enc_top = np.asarray(enc_top, np.float32)
    W_down = np.asarray(W_down, np.float32)
    E = np.asarray(encoder_weights, np.float32)

    # natural-E block: rows g*128+p, cols ck*64+m = E[g, ck*128+p, m]
    encnat = np.ascontiguousarray(
        E.reshape(NE, ACHUNK, P, SUB).transpose(0, 2, 1, 3)
    ).reshape(NE * P, E_COLS)
    # E^T table: rows g*64+s, cols a = E[g, a, s]
    tabB = np.ascontiguousarray(E.transpose(0, 2, 1)).reshape(NE * SUB, ATOMS)

    Wr = W_down.reshape(NE, SUB, NCHUNK, P)          # [g, m, j, p]
    Er = enc_top.reshape(NE, NCHUNK, P)              # [g, j, p]

    per_half = {}
    for h in (0, 1):
        order = _chunk_order(h)
        tabW = np.ascontiguousarray(
            Wr[:, :, order, :].transpose(0, 3, 2, 1)  # [g, p, jj, m]
        ).reshape(NE * P, W_COLS)
        encrow = (
            Er[:, order, :].transpose(0, 2, 1)        # [g, p, jj]
        ).reshape(NE * P, R_COLS)
        tabC = np.concatenate([encnat, encrow], axis=1)

        x_pm = np.ascontiguousarray(
            x.reshape(NCHUNK, P)[order, :].T)          # [p, jj]
        encbf = np.ascontiguousarray(
            Er[:, order, :].transpose(2, 1, 0)         # [p, jj, g]
        ).astype(ml_dtypes.bfloat16)
        per_half[h] = dict(
            tabw=tabW,
            tabc=tabC,
            cf32=x_pm,
            xbf=x_pm.astype(ml_dtypes.bfloat16),
            encbf=encbf,
        )

    in_maps = []
    for c in range(N_CORES):
        h, slot = c // 4, c % 4
        ph = per_half[h]
        cbf = np.zeros((P, NCHUNK + 8), ml_dtypes.bfloat16)
        cbf[:, :NCHUNK] = ph["xbf"]
        cbf[0, NCHUNK + slot] = 1.0
        in_maps.append({
            "tabw": ph["tabw"],
            "tabb": tabB,
            "tabc": ph["tabc"],
            "encbf": ph["encbf"],
            "cf32": ph["cf32"],
            "cbf16": cbf,
        })
    return in_maps


def _assemble(results):
    out = np.zeros(IN_DIM, np.float32).reshape(NCHUNK, P)
    for c in range(N_CORES):
        h = c // 4
        own = _chunk_order(h)[:HALF]
        out[own, :] += results[c]["out"].T
    return out.reshape(IN_DIM)


_NC_CACHE = {}
LAST_RESULT = {}


def kernel(x, enc_top, W_down, encoder_weights):
    in_maps = _host_prep(x, enc_top, W_down, encoder_weights)
    if "nc" not in _NC_CACHE:
        _NC_CACHE["nc"] = build_program()
    nc = _NC_CACHE["nc"]

    if os.environ.get("BASS_SIM") == "1":
        from concourse.bass_interp import CoreSim
        sim_cores = os.environ.get("BASS_SIM_CORES")
        cores = (
            [int(t) for t in sim_cores.split(",")] if sim_cores
            else range(N_CORES)
        )
        results = [None] * N_CORES
        for c in cores:
            nc_c = build_program()
            sim = CoreSim(nc_c)
            for name, arr in in_maps[c].items():
                sim.tensor(name)[:] = arr
            sim.simulate()
            results[c] = {"out": np.array(sim.tensor("out"))}
        for c in range(N_CORES):
            if results[c] is None:
                results[c] = {"out": np.zeros((P, HALF), np.float32)}
        return _assemble(results)

    trace = os.environ.get("BASS_TRACE") == "1"
    if trace:
        _ensure_trace_hook()
    res = run_bass_kernel_spmd(
        nc, in_maps, core_ids=list(range(N_CORES)),
        trace=trace,
    )
    LAST_RESULT["res"] = res
    return _assemble(res.results)


def _ensure_trace_hook():
    """Install the axon NTFF profile hook if antenv.axon_hooks is absent."""
    try:
        from antenv.axon_hooks import get_axon_ntff_profile_hook  # noqa
        return
    except ImportError:
        pass
    import sys
    import types
    try:
        from trn_agent_boot.trn_boot import _ntff_profile_via_ctypes
    except ImportError:
        return
    hook = _ntff_profile_via_ctypes("/opt/axon/libaxon_pjrt.so")
    mod = types.ModuleType("antenv.axon_hooks")
    mod._hook = hook
    mod.get_axon_ntff_profile_hook = lambda: mod._hook
    mod.set_axon_ntff_profile_hook = lambda h: setattr(mod, "_hook", h)
    import antenv
    sys.modules["antenv.axon_hooks"] = mod
    antenv.axon_hooks = mod


if __name__ == "__main__":
    nc = build_program()
    print("program built ok")


# revision 34
# speedup vs baseline: 1.0331x; 1.0051x over previous
"""Trainium2 Bass kernel for single-token MoE routing (nn_MixtureOfExperts_v2).

Problem:
    x [2304]; enc_top [256, 2304]; W_down [256, 64, 2304]; encoder_weights
    [256, 512, 64].
    codes = relu_offset(enc_top @ x)           (slope 0.0, offset 1/48)
    top4 values/indices of codes
    per selected expert i (gate v):
        s = W_down[i] @ x                      [64]
        c = relu_offset(E[i] @ s, slope 0.01)  [512]
        d = E[i]^T @ c                         [64]
        recon += W_down[i]^T @ d               [2304]
        recon += v * enc_top[i]
    output = recon                             [2304]

Distribution (8 cores, no collectives):
    Every core loads a replicated bf16 transposed copy of enc_top, computes
    all 256 codes on the PE, and runs top-4 on the vector engine
    (max_with_indices), so all cores agree on the routing.  Core c then
    processes selected slot (c % 4) alone: it gathers that expert's weights
    from a per-core table in HBM with one indirect DMA and runs the expert
    pipeline.  Cores c and c+4 process the same slot but emit complementary
    halves of the 2304-dim reconstruction (the per-core tables are built
    with the core's half of the input-dim chunks first, so the program is
    identical across cores - pure SPMD with per-core constants).  The host
    sums the 8 partial outputs (the cross-core reduction is a plain "+"
    done during unsharding).

Numerics: only the routing phase (codes -> top-4 indices) runs in bf16; the
top-4 gate values are recomputed in fp32 from gathered fp32 enc_top rows.
The entire expert pipeline is fp32 with fp32 PSUM accumulation.
"""

import os

import numpy as np
import ml_dtypes

import concourse.bacc as bacc
import concourse.bass as bass
import concourse.mybir as mybir
import concourse.tile as tile
from concourse.bass import IndirectOffsetOnAxis
from concourse.bass_utils import run_bass_kernel_spmd

# ---- problem constants (hardcoded per harness contract) ----
IN_DIM = 2304
SUB = 64
ATOMS = 512
NE = 256
K = 4
P = 128
NCHUNK = IN_DIM // P          # 18 chunks of 128 along input dim
HALF = NCHUNK // 2            # 9 chunks per core-half
ACHUNK = ATOMS // P           # 4 chunks of 128 along atoms
N_CORES = 8

W_COLS = NCHUNK * SUB         # 1152: W_down^T block (chunk-major, m innermost)
E_COLS = ACHUNK * SUB         # 256:  natural E block (atom-chunk-major)
R_COLS = NCHUNK               # 18:   enc_top row (chunk-major)
ET_COLS = ATOMS               # 512:  E^T block (rows 0..63 only, rest zero)
TABA_COLS = W_COLS + E_COLS + R_COLS + ET_COLS   # 1938
ET_OFF = W_COLS + E_COLS + R_COLS                # 1426
R_OFF = W_COLS + E_COLS                          # 1408

ENC_GROUPS = [3, 6, 9]        # enc_top chunks per DMA group (first smallest
NGRP = len(ENC_GROUPS)        # so the PE can start earliest)
N_WARM = int(os.environ.get("KERNEL_WARM_MMS", "64"))

OFFSET = float(np.float32(1.0) / np.float32(48.0))  # 1/sqrt(2304), fp32

F32 = mybir.dt.float32
BF16 = mybir.dt.bfloat16
I32 = mybir.dt.int32
U32 = mybir.dt.uint32


def build_program():
    nc = bacc.Bacc("TRN2", target_bir_lowering=False, debug=False,
                   enable_partition_id=False)

    tabW = nc.dram_tensor("tabw", [NE * P, W_COLS], F32,
                          kind="ExternalInput")
    tabB = nc.dram_tensor("tabb", [NE * P, ATOMS], F32,
                          kind="ExternalInput")
    tabC = nc.dram_tensor("tabc", [NE * P, E_COLS + R_COLS], F32,
                          kind="ExternalInput")
    encbf = nc.dram_tensor("encbf", [P, NCHUNK, NE], BF16,
                           kind="ExternalInput")
    # fp32 consts: cols 0:18 x (partition-major chunks); col 18 row0-7: unused
    cf32 = nc.dram_tensor("cf32", [P, NCHUNK], F32, kind="ExternalInput")
    # bf16 consts: cols 0:18 x; cols 18:26 row 0: slot one-hot
    cbf16 = nc.dram_tensor("cbf16", [P, NCHUNK + 8], BF16,
                           kind="ExternalInput")
    out_d = nc.dram_tensor("out", [P, HALF], F32, kind="ExternalOutput")

    with tile.TileContext(nc) as tc:
        with (
            tc.tile_pool(name="sb", bufs=1) as sb,
            tc.tile_pool(name="enc", bufs=1) as encp,
            tc.tile_pool(name="ps", bufs=1, space="PSUM") as ps,
        ):
            # ---- phase A: codes = enc_top @ x (bf16, PE) ----
            # first (smallest) enc group + consts on the scalar (ACT) queue,
            # remaining groups on the sync (SP) queue - parallel issue.
            enc_ts = []
            g0 = 0
            for gi, gn in enumerate(ENC_GROUPS):
                enc_t = encp.tile([P, gn, NE], BF16, tag=f"enc{gi}")
                nc.sync.dma_start(enc_t[:], encbf[:, g0:g0 + gn, :])
                enc_ts.append((enc_t, g0, gn))
                g0 += gn
            cb = sb.tile([P, NCHUNK + 8], BF16, tag="cbf")
            nc.scalar.dma_start(cb[:], cbf16[:])
            x_bf = cb[:, 0:NCHUNK]
            oh_bf = cb[0:1, NCHUNK:NCHUNK + 8]
            x_pm = sb.tile([P, NCHUNK], F32, tag="xpm")
            nc.scalar.dma_start(x_pm[:], cf32[:])

            # on-device constants
            ones_rbf = sb.tile([1, P], BF16, tag="onesrbf")
            nc.vector.memset(ones_rbf[:], 1.0)
            ones_c = sb.tile([P, 1], F32, tag="onesc")
            nc.vector.memset(ones_c[:], 1.0)

            codes_ps = ps.tile([1, NE], F32, tag="codes")
            for enc_t, g0, gn in enc_ts:
                for jo in range(gn):
                    jj = g0 + jo
                    nc.tensor.matmul(
                        codes_ps[:],
                        lhsT=x_bf[:, jj:jj + 1],
                        rhs=enc_t[:, jo, :],
                        start=(jj == 0),
                        stop=(jj == NCHUNK - 1),
                    )

            # ---- phase B: top-k (max8 on DVE, reading PSUM) + slot pick ----
            vals = sb.tile([1, 8], F32, tag="vals")
            idxs = sb.tile([1, 8], U32, tag="idxs")
            nc.vector.max_with_indices(vals[:], idxs[:], codes_ps[:])
            idxbf = sb.tile([1, 8], BF16, tag="idxbf")
            nc.vector.tensor_copy(idxbf[:], idxs[:])
            scr8 = sb.tile([1, 8], BF16, tag="scr8")
            nc.vector.tensor_tensor(
                out=scr8[:], in0=idxbf[:], in1=oh_bf,
                op=mybir.AluOpType.mult,
            )
            i_sel = sb.tile([1, 1], BF16, tag="isel")
            with nc.allow_low_precision(
                    reason="one-hot dot on small ints; exact in bf16"):
                nc.vector.tensor_reduce(
                    out=i_sel[:], in_=scr8[:], axis=mybir.AxisListType.X,
                    op=mybir.AluOpType.add,
                )
            # broadcast index to all partitions (bf16 single-pass matmul)
            ib_ps = ps.tile([P, 1], F32, tag="ib")
            nc.tensor.matmul(ib_ps[:], lhsT=ones_rbf[:], rhs=i_sel[:],
                             start=True, stop=True)
            iota_f = sb.tile([P, 1], F32, tag="iotaf")
            nc.gpsimd.iota(iota_f[:], pattern=[[0, 1]], base=0,
                           channel_multiplier=1,
                           allow_small_or_imprecise_dtypes=True)
            offa = sb.tile([P, 1], I32, tag="offa")
            nc.vector.scalar_tensor_tensor(
                out=offa[:], in0=ib_ps[:], scalar=float(P), in1=iota_f[:],
                op0=mybir.AluOpType.mult, op1=mybir.AluOpType.add,
            )

            # ---- phase C: gather this slot's expert blocks ----
            # W first (the s-step long pole), then E^T (c-step), then
            # E-natural + enc_top row (dT / gate-value steps).
            gW = sb.tile([P, W_COLS], F32, tag="gw")
            nc.gpsimd.indirect_dma_start(
                out=gW[:], out_offset=None, in_=tabW[:],
                in_offset=IndirectOffsetOnAxis(ap=offa[:, :1], axis=0),
            )
            gB = sb.tile([P, ATOMS], F32, tag="gb")
            nc.gpsimd.indirect_dma_start(
                out=gB[:], out_offset=None, in_=tabB[:],
                in_offset=IndirectOffsetOnAxis(ap=offa[:, :1], axis=0),
            )
            gC = sb.tile([P, E_COLS + R_COLS], F32, tag="gc")
            nc.gpsimd.indirect_dma_start(
                out=gC[:], out_offset=None, in_=tabC[:],
                in_offset=IndirectOffsetOnAxis(ap=offa[:, :1], axis=0),
            )

            # ---- PE warm-keeper: junk matmuls spanning the topk+gather
            # gap so HAM doesn't re-throttle the PE before the expert
            # pipeline (results written to a scratch PSUM bank, unused) ----
            if N_WARM:
                junk_ps = ps.tile([1, NE], F32, tag="junk")
                last_t, _, last_n = enc_ts[-1]
                for w in range(N_WARM):
                    nc.tensor.matmul(
                        junk_ps[:],
                        lhsT=x_bf[:, 0:1],
                        rhs=last_t[:, last_n - 1, :],
                        start=(w == 0),
                        stop=(w == N_WARM - 1),
                    )

            # ---- phase D: expert pipeline (fp32) ----
            # s = W @ x : accumulate over 18 chunks
            s_ps = ps.tile([SUB, 1], F32, tag="s")
            for jj in range(NCHUNK):
                nc.tensor.matmul(
                    s_ps[:],
                    lhsT=gW[:, jj * SUB:(jj + 1) * SUB],
                    rhs=x_pm[:, jj:jj + 1],
                    start=(jj == 0),
                    stop=(jj == NCHUNK - 1),
                )
            s_sb = sb.tile([SUB, 1], F32, tag="ssb")
            nc.vector.tensor_copy(s_sb[:], s_ps[:])

            # c = E @ s : 4 chunks of 128 atoms (lhsT = E^T slabs)
            c_ps = ps.tile([P, ACHUNK], F32, tag="c")
            for ck in range(ACHUNK):
                nc.tensor.matmul(
                    c_ps[:, ck:ck + 1],
                    lhsT=gB[0:SUB, ck * P:(ck + 1) * P],
                    rhs=s_sb[:],
                    start=True, stop=True,
                )
            # leaky relu with offset: c >= off ? c : 0.01*c
            cmask = sb.tile([P, ACHUNK], U32, tag="cmask")
            nc.vector.tensor_scalar(
                out=cmask[:], in0=c_ps[:], scalar1=OFFSET, scalar2=None,
                op0=mybir.AluOpType.is_ge,
            )
            cleak = sb.tile([P, ACHUNK], F32, tag="cleak")
            nc.vector.tensor_scalar(
                out=cleak[:], in0=c_ps[:], scalar1=0.01, scalar2=None,
                op0=mybir.AluOpType.mult,
            )
            c_relu = sb.tile([P, ACHUNK], F32, tag="crelu")
            nc.vector.select(c_relu[:], cmask[:], c_ps[:], cleak[:])

            # d^T = c^T @ E : accumulate 4 atom chunks -> [1, 64]
            dT_ps = ps.tile([1, SUB], F32, tag="dt")
            for ck in range(ACHUNK):
                nc.tensor.matmul(
                    dT_ps[:],
                    lhsT=c_relu[:, ck:ck + 1],
                    rhs=gC[:, ck * SUB:(ck + 1) * SUB],
                    start=(ck == 0),
                    stop=(ck == ACHUNK - 1),
                )

            # v = relu_offset(enc_top[i] . x) in fp32 (runs on DVE, parallel
            # with the PE chain above)
            vscr = sb.tile([P, NCHUNK], F32, tag="vscr")
            nc.vector.tensor_tensor(
                out=vscr[:], in0=gC[:, E_COLS:E_COLS + NCHUNK],
                in1=x_pm[:], op=mybir.AluOpType.mult,
            )
            vtmp = sb.tile([P, 1], F32, tag="vtmp")
            nc.vector.tensor_reduce(
                out=vtmp[:], in_=vscr[:], axis=mybir.AxisListType.X,
                op=mybir.AluOpType.add,
            )
            v_ps = ps.tile([1, 1], F32, tag="v")
            nc.tensor.matmul(v_ps[:], lhsT=vtmp[:], rhs=ones_c[:],
                             start=True, stop=True)
            vmask = sb.tile([1, 1], F32, tag="vmask")
            nc.vector.tensor_scalar(
                out=vmask[:], in0=v_ps[:], scalar1=OFFSET, scalar2=None,
                op0=mybir.AluOpType.is_ge,
            )

            # fused [d | v] broadcast to all partitions in one matmul pair
            dtv = sb.tile([1, SUB + 1], F32, tag="dtv")
            nc.vector.tensor_copy(dtv[:, 0:SUB], dT_ps[:])
            nc.vector.tensor_tensor(
                out=dtv[:, SUB:SUB + 1], in0=v_ps[:], in1=vmask[:],
                op=mybir.AluOpType.mult,
            )
            ones_r = sb.tile([1, P], F32, tag="onesr")
            nc.vector.memset(ones_r[:], 1.0)
            bb_ps = ps.tile([P, SUB + 1], F32, tag="bb")
            nc.tensor.matmul(bb_ps[:], lhsT=ones_r[:], rhs=dtv[:],
                             start=True, stop=True)

            # recon half: [128, 9] ; recon[p, jj] = sum_m W^T[p, jj, m]*d[m]
            prod = sb.tile([P, HALF, SUB], F32, tag="prod")
            gA_w3 = gW[:, 0:HALF * SUB].rearrange("p (j m) -> p j m", m=SUB)
            db_bc = bb_ps[:, None, 0:SUB].to_broadcast([P, HALF, SUB])
            nc.vector.tensor_tensor(
                out=prod[:], in0=gA_w3, in1=db_bc, op=mybir.AluOpType.mult,
            )
            recon = sb.tile([P, HALF], F32, tag="recon")
            nc.vector.tensor_reduce(
                out=recon[:], in_=prod[:], axis=mybir.AxisListType.X,
                op=mybir.AluOpType.add,
            )

            # final = recon + v * enc_row[:, :9]
            final = sb.tile([P, HALF], F32, tag="final")
            nc.vector.scalar_tensor_tensor(
                out=final[:],
                in0=gC[:, E_COLS:E_COLS + HALF],
                scalar=bb_ps[:, SUB:SUB + 1],
                in1=recon[:],
                op0=mybir.AluOpType.mult, op1=mybir.AluOpType.add,
            )
            nc.sync.dma_start(out_d[:], final[:])

    nc.compile()
    return nc


def _chunk_order(h):
    """Chunk visit order for core-half h: own half first."""
    own = list(range(h * HALF, (h + 1) * HALF))
    other = list(range((1 - h) * HALF, (2 - h) * HALF))
    return own + other


def _host_prep(x, enc_top, W_down, encoder_weights):
    """Build per-core-half input tables (pure layout transforms)."""
    x = np.asarray(x, np.float32)
    enc_top = np.asarray(enc_top, np.float32)
    W_down = np.asarray(W_down, np.float32)
    E = np.asarray(encoder_weights, np.float32)

    # natural-E block: rows g*128+p, cols ck*64+m = E[g, ck*128+p, m]
    encnat = np.ascontiguousarray(
        E.reshape(NE, ACHUNK, P, SUB).transpose(0, 2, 1, 3)
    ).reshape(NE * P, E_COLS)
    # E^T table: rows g*128+s (s<64; rest zero), cols a = E[g, a, s]
    tabB = np.zeros((NE, P, ATOMS), np.float32)
    tabB[:, :SUB, :] = E.transpose(0, 2, 1)
    tabB = tabB.reshape(NE * P, ATOMS)

    Wr = W_down.reshape(NE, SUB, NCHUNK, P)          # [g, m, j, p]
    Er = enc_top.reshape(NE, NCHUNK, P)              # [g, j, p]

    per_half = {}
    for h in (0, 1):
        order = _chunk_order(h)
        tabW = np.ascontiguousarray(
            Wr[:, :, order, :].transpose(0, 3, 2, 1)  # [g, p, jj, m]
        ).reshape(NE * P, W_COLS)
        encrow = (
            Er[:, order, :].transpose(0, 2, 1)        # [g, p, jj]
        ).reshape(NE * P, R_COLS)
        tabC = np.concatenate([encnat, encrow], axis=1)

        x_pm = np.ascontiguousarray(
            x.reshape(NCHUNK, P)[order, :].T)          # [p, jj]
        encbf = np.ascontiguousarray(
            Er[:, order, :].transpose(2, 1, 0)         # [p, jj, g]
        ).astype(ml_dtypes.bfloat16)
        per_half[h] = dict(
            tabw=tabW,
            tabc=tabC,
            cf32=x_pm,
            xbf=x_pm.astype(ml_dtypes.bfloat16),
            encbf=encbf,
        )

    in_maps = []
    for c in range(N_CORES):
        h, slot = c // 4, c % 4
        ph = per_half[h]
        cbf = np.zeros((P, NCHUNK + 8), ml_dtypes.bfloat16)
        cbf[:, :NCHUNK] = ph["xbf"]
        cbf[0, NCHUNK + slot] = 1.0
        in_maps.append({
            "tabw": ph["tabw"],
            "tabb": tabB,
            "tabc": ph["tabc"],
            "encbf": ph["encbf"],
            "cf32": ph["cf32"],
            "cbf16": cbf,
        })
    return in_maps


def _assemble(results):
    out = np.zeros(IN_DIM, np.float32).reshape(NCHUNK, P)
    for c in range(N_CORES):
        h = c // 4
        own = _chunk_order(h)[:HALF]
        out[own, :] += results[c]["out"].T
    return out.reshape(IN_DIM)


_NC_CACHE = {}
LAST_RESULT = {}


def kernel(x, enc_top, W_down, encoder_weights):
    in_maps = _host_prep(x, enc_top, W_down, encoder_weights)
    if "nc" not in _NC_CACHE:
        _NC_CACHE["nc"] = build_program()
    nc = _NC_CACHE["nc"]

    if os.environ.get("BASS_SIM") == "1":
        from concourse.bass_interp import CoreSim
        sim_cores = os.environ.get("BASS_SIM_CORES")
        cores = (
            [int(t) for t in sim_cores.split(",")] if sim_cores
            else range(N_CORES)
        )
        results = [None] * N_CORES
        for c in cores:
            nc_c = build_program()
            sim = CoreSim(nc_c)
            for name, arr in in_maps[c].items():
                sim.tensor(name)[:] = arr
            sim.simulate()
            results[c] = {"out": np.array(sim.tensor("out"))}
        for c in range(N_CORES):
            if results[c] is None:
                results[c] = {"out": np.zeros((P, HALF), np.float32)}
        return _assemble(results)

    trace = os.environ.get("BASS_TRACE") == "1"
    if trace:
        _ensure_trace_hook()
    res = run_bass_kernel_spmd(
        nc, in_maps, core_ids=list(range(N_CORES)),
        trace=trace,
    )
    LAST_RESULT["res"] = res
    return _assemble(res.results)


def _ensure_trace_hook():
    """Install the axon NTFF profile hook if antenv.axon_hooks is absent."""
    try:
        from antenv.axon_hooks import get_axon_ntff_profile_hook  # noqa
        return
    except ImportError:
        pass
    import sys
    import types
    try:
        from trn_agent_boot.trn_boot import _ntff_profile_via_ctypes
    except ImportError:
        return
    hook = _ntff_profile_via_ctypes("/opt/axon/libaxon_pjrt.so")
    mod = types.ModuleType("antenv.axon_hooks")
    mod._hook = hook
    mod.get_axon_ntff_profile_hook = lambda: mod._hook
    mod.set_axon_ntff_profile_hook = lambda h: setattr(mod, "_hook", h)
    import antenv
    sys.modules["antenv.axon_hooks"] = mod
    antenv.axon_hooks = mod


if __name__ == "__main__":
    nc = build_program()
    print("program built ok")


# revision 37
# speedup vs baseline: 1.0915x; 1.0566x over previous
"""Trainium2 Bass kernel for single-token MoE routing (nn_MixtureOfExperts_v2).

Problem:
    x [2304]; enc_top [256, 2304]; W_down [256, 64, 2304]; encoder_weights
    [256, 512, 64].
    codes = relu_offset(enc_top @ x)           (slope 0.0, offset 1/48)
    top4 values/indices of codes
    per selected expert i (gate v):
        s = W_down[i] @ x                      [64]
        c = relu_offset(E[i] @ s, slope 0.01)  [512]
        d = E[i]^T @ c                         [64]
        recon += W_down[i]^T @ d               [2304]
        recon += v * enc_top[i]
    output = recon                             [2304]

Distribution (8 cores, no collectives):
    Every core loads a replicated bf16 transposed copy of enc_top, computes
    all 256 codes on the PE, and runs top-4 on the vector engine
    (max_with_indices), so all cores agree on the routing.  Core c then
    processes selected slot (c % 4) alone: it gathers that expert's weights
    from a per-core table in HBM with one indirect DMA and runs the expert
    pipeline.  Cores c and c+4 process the same slot but emit complementary
    halves of the 2304-dim reconstruction (the per-core tables are built
    with the core's half of the input-dim chunks first, so the program is
    identical across cores - pure SPMD with per-core constants).  The host
    sums the 8 partial outputs (the cross-core reduction is a plain "+"
    done during unsharding).

Numerics: only the routing phase (codes -> top-4 indices) runs in bf16; the
top-4 gate values are recomputed in fp32 from gathered fp32 enc_top rows.
The entire expert pipeline is fp32 with fp32 PSUM accumulation.
"""

import os

import numpy as np
import ml_dtypes

import concourse.bacc as bacc
import concourse.bass as bass
import concourse.mybir as mybir
import concourse.tile as tile
from concourse.bass import IndirectOffsetOnAxis
from concourse.bass_utils import run_bass_kernel_spmd

# ---- problem constants (hardcoded per harness contract) ----
IN_DIM = 2304
SUB = 64
ATOMS = 512
NE = 256
K = 4
P = 128
NCHUNK = IN_DIM // P          # 18 chunks of 128 along input dim
HALF = NCHUNK // 2            # 9 chunks per core-half
ACHUNK = ATOMS // P           # 4 chunks of 128 along atoms
N_CORES = 8

W_COLS = NCHUNK * SUB         # 1152: W_down^T block (chunk-major, m innermost)
E_COLS = ACHUNK * SUB         # 256:  natural E block (atom-chunk-major)
R_COLS = NCHUNK               # 18:   enc_top row (chunk-major)
ET_COLS = ATOMS               # 512:  E^T block (rows 0..63 only, rest zero)
TABA_COLS = W_COLS + E_COLS + R_COLS + ET_COLS   # 1938
ET_OFF = W_COLS + E_COLS + R_COLS                # 1426
R_OFF = W_COLS + E_COLS                          # 1408

ENC_GROUPS = [2, 4, 6, 6]     # enc_top chunks per DMA group (first smallest
NGRP = len(ENC_GROUPS)        # so the PE can start earliest)
N_WARM = int(os.environ.get("KERNEL_WARM_MMS", "64"))
N_PREWARM = int(os.environ.get("KERNEL_PREWARM_MMS", "26"))

OFFSET = float(np.float32(1.0) / np.float32(48.0))  # 1/sqrt(2304), fp32

F32 = mybir.dt.float32
BF16 = mybir.dt.bfloat16
I32 = mybir.dt.int32
U32 = mybir.dt.uint32


def build_program():
    nc = bacc.Bacc("TRN2", target_bir_lowering=False, debug=False,
                   enable_partition_id=False)

    tabW = nc.dram_tensor("tabw", [NE * P, W_COLS], F32,
                          kind="ExternalInput")
    tabB = nc.dram_tensor("tabb", [NE * P, ATOMS], F32,
                          kind="ExternalInput")
    tabC = nc.dram_tensor("tabc", [NE * P, E_COLS + R_COLS], F32,
                          kind="ExternalInput")
    encbf = nc.dram_tensor("encbf", [P, NCHUNK, NE], BF16,
                           kind="ExternalInput")
    # fp32 consts: cols 0:18 x (partition-major chunks); col 18 row0-7: unused
    cf32 = nc.dram_tensor("cf32", [P, NCHUNK], F32, kind="ExternalInput")
    # bf16 consts: cols 0:18 x; cols 18:26 row 0: slot one-hot
    cbf16 = nc.dram_tensor("cbf16", [P, NCHUNK + 8], BF16,
                           kind="ExternalInput")
    out_d = nc.dram_tensor("out", [P, HALF], F32, kind="ExternalOutput")

    with tile.TileContext(nc) as tc:
        with (
            tc.tile_pool(name="sb", bufs=1) as sb,
            tc.tile_pool(name="enc", bufs=1) as encp,
            tc.tile_pool(name="ps", bufs=1, space="PSUM") as ps,
        ):
            # ---- phase A: codes = enc_top @ x (bf16, PE) ----
            # first (smallest) enc group + consts on the scalar (ACT) queue,
            # remaining groups on the sync (SP) queue - parallel issue.
            enc_ts = []
            g0 = 0
            for gi, gn in enumerate(ENC_GROUPS):
                enc_t = encp.tile([P, gn, NE], BF16, tag=f"enc{gi}")
                nc.sync.dma_start(enc_t[:], encbf[:, g0:g0 + gn, :])
                enc_ts.append((enc_t, g0, gn))
                g0 += gn
            cb = sb.tile([P, NCHUNK + 8], BF16, tag="cbf")
            nc.scalar.dma_start(cb[:], cbf16[:])
            x_bf = cb[:, 0:NCHUNK]
            oh_bf = cb[0:1, NCHUNK:NCHUNK + 8]
            x_pm = sb.tile([P, NCHUNK], F32, tag="xpm")
            nc.scalar.dma_start(x_pm[:], cf32[:])

            # on-device constants
            ones_rbf = sb.tile([1, P], BF16, tag="onesrbf")
            nc.vector.memset(ones_rbf[:], 1.0)
            ones_c = sb.tile([P, 1], F32, tag="onesc")
            nc.vector.memset(ones_c[:], 1.0)

            # ---- PE pre-warm: matmuls on a zeroed tile while the first
            # enc-group DMA is in flight, so HAM un-throttles the PE to
            # 2.4 GHz before the codes matmuls start ----
            junk_ps = ps.tile([1, NE], F32, tag="junk")
            if N_PREWARM:
                zwarm = sb.tile([P, P], BF16, tag="zwarm")
                nc.vector.memset(zwarm[:], 0.0)
                for w in range(N_PREWARM):
                    nc.tensor.matmul(
                        junk_ps[:, 0:P],
                        lhsT=zwarm[:, 0:1],
                        rhs=zwarm[:],
                        start=(w == 0),
                        stop=(w == N_PREWARM - 1),
                    )

            codes_ps = ps.tile([1, NE], F32, tag="codes")
            for enc_t, g0, gn in enc_ts:
                for jo in range(gn):
                    jj = g0 + jo
                    nc.tensor.matmul(
                        codes_ps[:],
                        lhsT=x_bf[:, jj:jj + 1],
                        rhs=enc_t[:, jo, :],
                        start=(jj == 0),
                        stop=(jj == NCHUNK - 1),
                    )

            # ---- phase B: top-k (max8 on DVE, reading PSUM) + slot pick ----
            vals = sb.tile([1, 8], F32, tag="vals")
            idxs = sb.tile([1, 8], U32, tag="idxs")
            nc.vector.max_with_indices(vals[:], idxs[:], codes_ps[:])
            idxbf = sb.tile([1, 8], BF16, tag="idxbf")
            nc.vector.tensor_copy(idxbf[:], idxs[:])
            scr8 = sb.tile([1, 8], BF16, tag="scr8")
            nc.vector.tensor_tensor(
                out=scr8[:], in0=idxbf[:], in1=oh_bf,
                op=mybir.AluOpType.mult,
            )
            i_sel = sb.tile([1, 1], BF16, tag="isel")
            with nc.allow_low_precision(
                    reason="one-hot dot on small ints; exact in bf16"):
                nc.vector.tensor_reduce(
                    out=i_sel[:], in_=scr8[:], axis=mybir.AxisListType.X,
                    op=mybir.AluOpType.add,
                )
            # broadcast index to all partitions (bf16 single-pass matmul)
            ib_ps = ps.tile([P, 1], F32, tag="ib")
            nc.tensor.matmul(ib_ps[:], lhsT=ones_rbf[:], rhs=i_sel[:],
                             start=True, stop=True)
            iota_f = sb.tile([P, 1], F32, tag="iotaf")
            nc.gpsimd.iota(iota_f[:], pattern=[[0, 1]], base=0,
                           channel_multiplier=1,
                           allow_small_or_imprecise_dtypes=True)
            offa = sb.tile([P, 1], I32, tag="offa")
            nc.vector.scalar_tensor_tensor(
                out=offa[:], in0=ib_ps[:], scalar=float(P), in1=iota_f[:],
                op0=mybir.AluOpType.mult, op1=mybir.AluOpType.add,
            )

            # ---- phase C: gather this slot's expert blocks ----
            # W first (the s-step long pole), then E^T (c-step), then
            # E-natural + enc_top row (dT / gate-value steps).
            gW = sb.tile([P, W_COLS], F32, tag="gw")
            nc.gpsimd.indirect_dma_start(
                out=gW[:], out_offset=None, in_=tabW[:],
                in_offset=IndirectOffsetOnAxis(ap=offa[:, :1], axis=0),
            )
            gB = sb.tile([P, ATOMS], F32, tag="gb")
            nc.gpsimd.indirect_dma_start(
                out=gB[:], out_offset=None, in_=tabB[:],
                in_offset=IndirectOffsetOnAxis(ap=offa[:, :1], axis=0),
            )
            gC = sb.tile([P, E_COLS + R_COLS], F32, tag="gc")
            nc.gpsimd.indirect_dma_start(
                out=gC[:], out_offset=None, in_=tabC[:],
                in_offset=IndirectOffsetOnAxis(ap=offa[:, :1], axis=0),
            )

            # ---- PE warm-keeper: junk matmuls spanning the topk+gather
            # gap so HAM doesn't re-throttle the PE before the expert
            # pipeline (results written to a scratch PSUM bank, unused) ----
            if N_WARM:
                last_t, _, last_n = enc_ts[-1]
                for w in range(N_WARM):
                    nc.tensor.matmul(
                        junk_ps[:],
                        lhsT=x_bf[:, 0:1],
                        rhs=last_t[:, last_n - 1, :],
                        start=(w == 0),
                        stop=(w == N_WARM - 1),
                    )

            # ---- phase D: expert pipeline (fp32) ----
            # s = W @ x : accumulate over 18 chunks
            s_ps = ps.tile([SUB, 1], F32, tag="s")
            for jj in range(NCHUNK):
                nc.tensor.matmul(
                    s_ps[:],
                    lhsT=gW[:, jj * SUB:(jj + 1) * SUB],
                    rhs=x_pm[:, jj:jj + 1],
                    start=(jj == 0),
                    stop=(jj == NCHUNK - 1),
                )
            s_sb = sb.tile([SUB, 1], F32, tag="ssb")
            nc.vector.tensor_copy(s_sb[:], s_ps[:])

            # c = E @ s : 4 chunks of 128 atoms (lhsT = E^T slabs)
            c_ps = ps.tile([P, ACHUNK], F32, tag="c")
            for ck in range(ACHUNK):
                nc.tensor.matmul(
                    c_ps[:, ck:ck + 1],
                    lhsT=gB[0:SUB, ck * P:(ck + 1) * P],
                    rhs=s_sb[:],
                    start=True, stop=True,
                )
            # leaky relu with offset: c >= off ? c : 0.01*c
            cmask = sb.tile([P, ACHUNK], U32, tag="cmask")
            nc.vector.tensor_scalar(
                out=cmask[:], in0=c_ps[:], scalar1=OFFSET, scalar2=None,
                op0=mybir.AluOpType.is_ge,
            )
            cleak = sb.tile([P, ACHUNK], F32, tag="cleak")
            nc.vector.tensor_scalar(
                out=cleak[:], in0=c_ps[:], scalar1=0.01, scalar2=None,
                op0=mybir.AluOpType.mult,
            )
            c_relu = sb.tile([P, ACHUNK], F32, tag="crelu")
            nc.vector.select(c_relu[:], cmask[:], c_ps[:], cleak[:])

            # d^T = c^T @ E : accumulate 4 atom chunks -> [1, 64]
            dT_ps = ps.tile([1, SUB], F32, tag="dt")
            for ck in range(ACHUNK):
                nc.tensor.matmul(
                    dT_ps[:],
                    lhsT=c_relu[:, ck:ck + 1],
                    rhs=gC[:, ck * SUB:(ck + 1) * SUB],
                    start=(ck == 0),
                    stop=(ck == ACHUNK - 1),
                )

            # v = relu_offset(enc_top[i] . x) in fp32 (runs on DVE, parallel
            # with the PE chain above)
            vscr = sb.tile([P, NCHUNK], F32, tag="vscr")
            nc.vector.tensor_tensor(
                out=vscr[:], in0=gC[:, E_COLS:E_COLS + NCHUNK],
                in1=x_pm[:], op=mybir.AluOpType.mult,
            )
            vtmp = sb.tile([P, 1], F32, tag="vtmp")
            nc.vector.tensor_reduce(
                out=vtmp[:], in_=vscr[:], axis=mybir.AxisListType.X,
                op=mybir.AluOpType.add,
            )
            v_ps = ps.tile([1, 1], F32, tag="v")
            nc.tensor.matmul(v_ps[:], lhsT=vtmp[:], rhs=ones_c[:],
                             start=True, stop=True)
            vmask = sb.tile([1, 1], F32, tag="vmask")
            nc.vector.tensor_scalar(
                out=vmask[:], in0=v_ps[:], scalar1=OFFSET, scalar2=None,
                op0=mybir.AluOpType.is_ge,
            )

            # fused [d | v] broadcast to all partitions in one matmul pair
            dtv = sb.tile([1, SUB + 1], F32, tag="dtv")
            nc.vector.tensor_copy(dtv[:, 0:SUB], dT_ps[:])
            nc.vector.tensor_tensor(
                out=dtv[:, SUB:SUB + 1], in0=v_ps[:], in1=vmask[:],
                op=mybir.AluOpType.mult,
            )
            ones_r = sb.tile([1, P], F32, tag="onesr")
            nc.vector.memset(ones_r[:], 1.0)
            bb_ps = ps.tile([P, SUB + 1], F32, tag="bb")
            nc.tensor.matmul(bb_ps[:], lhsT=ones_r[:], rhs=dtv[:],
                             start=True, stop=True)

            # recon half: [128, 9] ; recon[p, jj] = sum_m W^T[p, jj, m]*d[m]
            prod = sb.tile([P, HALF, SUB], F32, tag="prod")
            gA_w3 = gW[:, 0:HALF * SUB].rearrange("p (j m) -> p j m", m=SUB)
            db_bc = bb_ps[:, None, 0:SUB].to_broadcast([P, HALF, SUB])
            nc.vector.tensor_tensor(
                out=prod[:], in0=gA_w3, in1=db_bc, op=mybir.AluOpType.mult,
            )
            recon = sb.tile([P, HALF], F32, tag="recon")
            nc.vector.tensor_reduce(
                out=recon[:], in_=prod[:], axis=mybir.AxisListType.X,
                op=mybir.AluOpType.add,
            )

            # final = recon + v * enc_row[:, :9]
            final = sb.tile([P, HALF], F32, tag="final")
            nc.vector.scalar_tensor_tensor(
                out=final[:],
                in0=gC[:, E_COLS:E_COLS + HALF],
                scalar=bb_ps[:, SUB:SUB + 1],
                in1=recon[:],
                op0=mybir.AluOpType.mult, op1=mybir.AluOpType.add,
            )
            nc.sync.dma_start(out_d[:], final[:])

    nc.compile()
    return nc


def _chunk_order(h):
    """Chunk visit order for core-half h: own half first."""
    own = list(range(h * HALF, (h + 1) * HALF))
    other = list(range((1 - h) * HALF, (2 - h) * HALF))
    return own + other


def _host_prep(x, enc_top, W_down, encoder_weights):
    """Build per-core-half input tables (pure layout transforms)."""
    x = np.asarray(x, np.float32)
    enc_top = np.asarray(enc_top, np.float32)
    W_down = np.asarray(W_down, np.float32)
    E = np.asarray(encoder_weights, np.float32)

    # natural-E block: rows g*128+p, cols ck*64+m = E[g, ck*128+p, m]
    encnat = np.ascontiguousarray(
        E.reshape(NE, ACHUNK, P, SUB).transpose(0, 2, 1, 3)
    ).reshape(NE * P, E_COLS)
    # E^T table: rows g*128+s (s<64; rest zero), cols a = E[g, a, s]
    tabB = np.zeros((NE, P, ATOMS), np.float32)
    tabB[:, :SUB, :] = E.transpose(0, 2, 1)
    tabB = tabB.reshape(NE * P, ATOMS)

    Wr = W_down.reshape(NE, SUB, NCHUNK, P)          # [g, m, j, p]
    Er = enc_top.reshape(NE, NCHUNK, P)              # [g, j, p]

    per_half = {}
    for h in (0, 1):
        order = _chunk_order(h)
        tabW = np.ascontiguousarray(
            Wr[:, :, order, :].transpose(0, 3, 2, 1)  # [g, p, jj, m]
        ).reshape(NE * P, W_COLS)
        encrow = (
            Er[:, order, :].transpose(0, 2, 1)        # [g, p, jj]
        ).reshape(NE * P, R_COLS)
        tabC = np.concatenate([encnat, encrow], axis=1)

        x_pm = np.ascontiguousarray(
            x.reshape(NCHUNK, P)[order, :].T)          # [p, jj]
        encbf = np.ascontiguousarray(
            Er[:, order, :].transpose(2, 1, 0)         # [p, jj, g]
        ).astype(ml_dtypes.bfloat16)
        per_half[h] = dict(
            tabw=tabW,
            tabc=tabC,
            cf32=x_pm,
            xbf=x_pm.astype(ml_dtypes.bfloat16),
            encbf=encbf,
        )

    in_maps = []
    for c in range(N_CORES):
        h, slot = c // 4, c % 4
        ph = per_half[h]
        cbf = np.zeros((P, NCHUNK + 8), ml_dtypes.bfloat16)
        cbf[:, :NCHUNK] = ph["xbf"]
        cbf[0, NCHUNK + slot] = 1.0
        in_maps.append({
            "tabw": ph["tabw"],
            "tabb": tabB,
            "tabc": ph["tabc"],
            "encbf": ph["encbf"],
            "cf32": ph["cf32"],
            "cbf16": cbf,
        })
    return in_maps


def _assemble(results):
    out = np.zeros(IN_DIM, np.float32).reshape(NCHUNK, P)
    for c in range(N_CORES):
        h = c // 4
        own = _chunk_order(h)[:HALF]
        out[own, :] += results[c]["out"].T
    return out.reshape(IN_DIM)


_NC_CACHE = {}
LAST_RESULT = {}


def kernel(x, enc_top, W_down, encoder_weights):
    in_maps = _host_prep(x, enc_top, W_down, encoder_weights)
    if "nc" not in _NC_CACHE:
        _NC_CACHE["nc"] = build_program()
    nc = _NC_CACHE["nc"]

    if os.environ.get("BASS_SIM") == "1":
        from concourse.bass_interp import CoreSim
        sim_cores = os.environ.get("BASS_SIM_CORES")
        cores = (
            [int(t) for t in sim_cores.split(",")] if sim_cores
            else range(N_CORES)
        )
        results = [None] * N_CORES
        for c in cores:
            nc_c = build_program()
            sim = CoreSim(nc_c)
            for name, arr in in_maps[c].items():
                sim.tensor(name)[:] = arr
            sim.simulate()
            results[c] = {"out": np.array(sim.tensor("out"))}
        for c in range(N_CORES):
            if results[c] is None:
                results[c] = {"out": np.zeros((P, HALF), np.float32)}
        return _assemble(results)

    trace = os.environ.get("BASS_TRACE") == "1"
    if trace:
        _ensure_trace_hook()
    res = run_bass_kernel_spmd(
        nc, in_maps, core_ids=list(range(N_CORES)),
        trace=trace,
    )
    LAST_RESULT["res"] = res
    return _assemble(res.results)


def _ensure_trace_hook():
    """Install the axon NTFF profile hook if antenv.axon_hooks is absent."""
    try:
        from antenv.axon_hooks import get_axon_ntff_profile_hook  # noqa
        return
    except ImportError:
        pass
    import sys
    import types
    try:
        from trn_agent_boot.trn_boot import _ntff_profile_via_ctypes
    except ImportError:
        return
    hook = _ntff_profile_via_ctypes("/opt/axon/libaxon_pjrt.so")
    mod = types.ModuleType("antenv.axon_hooks")
    mod._hook = hook
    mod.get_axon_ntff_profile_hook = lambda: mod._hook
    mod.set_axon_ntff_profile_hook = lambda h: setattr(mod, "_hook", h)
    import antenv
    sys.modules["antenv.axon_hooks"] = mod
    antenv.axon_hooks = mod


if __name__ == "__main__":
    nc = build_program()
    print("program built ok")
